# revision 1
# baseline (speedup 1.0000x reference)
"""Trainium2 Bass kernel for nn_LinearEncoder (gnn_message_passing).

Reference computes, for N=512 nodes with n_in = n_out = 256:
    i, j = triu_indices(N, k=1)
    edges = concat([x[i], x[j]], -1)            # [E, 512]
    h = edges @ W.T + b                         # [E, 256]
    out[i, j] = h ; out = out + out.T           # [N, N, 256], 0 diagonal

Key algebraic identity: with W = [W1 | W2],
    h(i, j) = A[i] + B[j] + b,   A = x @ W1.T,  B = x @ W2.T
so the full output is
    out[i, j] = A[min(i,j)] + B'[max(i,j)]      (B' = B + b), 0 on diagonal.

Sharding: output rows split across 8 cores (64 rows each), one SPMD
program.  Core k receives x pre-rotated by its row base
(x_rot[t] = x[(base+t) % 512]) so the triangular "diagonal block" sits at
local columns s in [0, 64) on every core; region selection (A vs B')
enters only through small 0/1 mask *inputs*.

Per row-pair rp (rows r0 = 2rp, r0+1), the device computes:
  - three 128-wide column blocks:  PSUM = masks.T @ row-table (bf16 hi+lo
    split, exact to ~2^-17), run CONCURRENTLY on the PE via distinct
    row-groups (tile_position), then evacuated by VectorE as
    sbuf = PSUM + ColTable_f32 (column terms exact fp32);
  - block0 upper half: same masked-broadcast + DVE fold;
  - the triangular diagonal block: two constant masked-selection matmul
    pairs (including the exact-zero diagonal), evacuated by ScalarE.
DMA streams ~33.5 MB/core of output to HBM — the roofline.
"""

import os
import sys

for _p in ("/opt/trn_rl_repo", "/root/.axon_site/_ro/trn_rl_repo"):
    if os.path.isdir(_p) and _p not in sys.path:
        sys.path.insert(0, _p)

import numpy as np
import ml_dtypes

import concourse.bass as bass
import concourse.bacc as bacc
import concourse.mybir as mybir
import concourse.tile as tile
from concourse.bass_utils import run_bass_kernel_spmd

N = 512
CH = 256          # n_out
NIN = 256         # n_in
NCORES = 8
RB = N // NCORES  # 64 rows per core
F32 = mybir.dt.float32
BF16 = mybir.dt.bfloat16
BF16NP = ml_dtypes.bfloat16


# --------------------------------------------------------------------------
# host-side constant builders
# --------------------------------------------------------------------------

def _masks_RL(k: int):
    """R/L region indicators over local columns s for core k."""
    base = RB * k
    wrap = N - base  # columns s >= wrap hold wrapped (j < base) entries
    s = np.arange(N)
    R = ((s >= 64) & (s < wrap)).astype(np.float32)
    L = (s >= wrap).astype(np.float32)
    return R, L


def _diag_consts():
    """Constant masked-selection weights for the 64x64 diagonal blocks.

    For row-pair rp, output column m = q*64 + s (q in {0,1}, s in [0,64)),
    with r_q = 2*rp + q and rhs = [A_rot[0:64] ; B'_rot[0:64]] (K = 128):
      L side (s < r_q):  value = A_rot[s] + B'_rot[r_q]
      R side (s > r_q):  value = B'_rot[s] + A_rot[r_q]
      s == r_q: all weights zero -> exact 0 output.
    """
    dl = np.zeros((128, 32 * 128), np.float32)
    dr = np.zeros((128, 32 * 128), np.float32)
    for rp in range(32):
        for q in range(2):
            r_q = 2 * rp + q
            for s in range(64):
                m = rp * 128 + q * 64 + s
                if s < r_q:
                    dl[s, m] = 1.0            # A_rot[s]
                    dl[64 + r_q, m] = 1.0     # B'_rot[r_q]
                elif s > r_q:
                    dr[64 + s, m] = 1.0       # B'_rot[s]
                    dr[r_q, m] = 1.0          # A_rot[r_q]
    return dl, dr


def _shared_inputs(W: np.ndarray, b: np.ndarray):
    W = np.asarray(W, np.float32)
    b = np.asarray(b, np.float32)
    dl, dr = _diag_consts()
    w12 = np.concatenate(
        [np.ascontiguousarray(W[:, :NIN].T), np.ascontiguousarray(W[:, NIN:].T)],
        axis=1)                                     # [in, 2*out] = [A | B]
    b2 = np.concatenate([np.zeros(CH, np.float32), b]).reshape(1, 2 * CH)
    return {
        "w12t": w12,
        "b2_row": b2,
        "diag_l": dl.astype(BF16NP),
        "diag_r": dr.astype(BF16NP),
    }


def _core_inputs(x: np.ndarray, k: int):
    x = np.asarray(x, np.float32)
    base = RB * k
    x_rot = np.roll(x, -base, axis=0)
    R, L = _masks_RL(k)

    cm = np.zeros((128, 8), np.float32)
    for t in range(4):
        cm[:, t] = R[128 * t:128 * (t + 1)]
        cm[:, 4 + t] = L[128 * t:128 * (t + 1)]

    # One [128, 512] lhsT tensor: the masked-broadcast weights for the
    # three main column blocks live in PE row-groups 0/1/2 (partitions
    # 0-3, 32-35, 64-67; rows = R, L, R, L over hi/lo flat tables) and
    # block0-upper's K=8 weights in row-group 3 (partitions 96-103) —
    # the four small-K matmuls then run concurrently on the PE.
    wm4 = np.stack([R, L, R, L])                        # [4, 512]
    wm0 = np.zeros((8, 128), np.float32)
    p = np.arange(64)
    wm0[0, :64] = R[64 + p]
    wm0[1, :64] = L[64 + p]
    wm0[2, :64] = R[64 + p]
    wm0[3, :64] = L[64 + p]
    wm0[4, 64:] = R[64 + p]
    wm0[5, 64:] = L[64 + p]
    wm0[6, 64:] = R[64 + p]
    wm0[7, 64:] = L[64 + p]
    wmbig = np.zeros((128, 512), np.float32)
    for gp in (0, 32, 64):
        wmbig[gp:gp + 4, :] = wm4
    wmbig[96:104, 0:128] = wm0
    return {
        "xt_rot": np.ascontiguousarray(x_rot.T),  # [in=256, node=512]
        "cm": cm,
        "wm": wmbig.astype(BF16NP),
    }


# --------------------------------------------------------------------------
# device program
# --------------------------------------------------------------------------

_PROGRAM = None


def _build_program() -> bass.Bass:
    nc = bacc.Bacc()
    f32 = F32
    npad = 68  # padded flat scratch rows

    # ---- dram tensors -----------------------------------------------------
    xt_rot = nc.dram_tensor("xt_rot", [NIN, N], f32, kind="ExternalInput")
    w12t = nc.dram_tensor("w12t", [NIN, 2 * CH], f32, kind="ExternalInput")
    b2_row = nc.dram_tensor("b2_row", [1, 2 * CH], f32, kind="ExternalInput")
    cm = nc.dram_tensor("cm", [128, 8], f32, kind="ExternalInput")
    d_wm = nc.dram_tensor("wm", [128, N], BF16, kind="ExternalInput")
    d_dl = nc.dram_tensor("diag_l", [128, 32 * 128], BF16, kind="ExternalInput")
    d_dr = nc.dram_tensor("diag_r", [128, 32 * 128], BF16, kind="ExternalInput")

    # DMA-native contiguous layouts; the host unpicks them (free).
    # slab_m[3g + J-1, p, (sub, q, ch)] = value(row 8g+2sub+q, s = 128J+p)
    # out0d/u[g, q*64+s, (sub, ch)]    = diag/upper block values
    slab_m = nc.dram_tensor("slab_m", [24, 128, 2048], f32,
                            kind="ExternalOutput")
    out0d = nc.dram_tensor("out0d", [8, 128, 1024], f32, kind="ExternalOutput")
    out0u = nc.dram_tensor("out0u", [8, 128, 1024], f32, kind="ExternalOutput")

    with tile.TileContext(nc) as tc:
        with (
            tc.tile_pool(name="const", bufs=1) as cpool,
            tc.tile_pool(name="tmp", bufs=3) as tpool,
            tc.tile_pool(name="psA", bufs=5, space="PSUM") as psA,
            tc.tile_pool(name="ps0", bufs=3, space="PSUM") as ps0,
            tc.tile_pool(name="stM", bufs=7) as stM,
            tc.tile_pool(name="st0", bufs=6) as st0,
        ):
            # ---- load inputs ---------------------------------------------
            def load(dram, shape, dtype, tag):
                t = cpool.tile(shape, dtype, tag=tag)
                nc.sync.dma_start(out=t[:], in_=dram[:])
                return t

            xt0 = load(xt_rot[0:128, :], [128, N], f32, "xt0")
            xt1 = load(xt_rot[128:256, :], [128, N], f32, "xt1")
            w12a = load(w12t[0:128, :], [128, 2 * CH], f32, "w12a")
            w12b = load(w12t[128:256, :], [128, 2 * CH], f32, "w12b")
            b2t = load(b2_row, [1, 2 * CH], f32, "b2t")
            cmt = load(cm, [128, 8], f32, "cmt")
            wmt = load(d_wm, [128, N], BF16, "wmt")
            dlt = cpool.tile([128, 32 * 128], BF16, tag="dlt")
            nc.gpsimd.dma_start(out=dlt[:], in_=d_dl[:])
            drt = cpool.tile([128, 32 * 128], BF16, tag="drt")
            nc.gpsimd.dma_start(out=drt[:], in_=d_dr[:])

            ones1 = cpool.tile([1, 128], f32, tag="ones1")
            nc.vector.memset(ones1[:], 1.0)

            # ---- phase 1: tables [A | B'] (one [128, 512] psum per s) ----
            A_t, Bp_t = [], []
            for s in range(4):
                pa = psA.tile([128, 2 * CH], f32, tag="pj", name=f"ptb{s}")
                mmd = nc.tensor.matmul
                mmd(pa[:], xt0[:, 128 * s:128 * (s + 1)], w12a[:],
                    start=True, stop=False)
                mmd(pa[:], xt1[:, 128 * s:128 * (s + 1)], w12b[:],
                    start=False, stop=False)
                mmd(pa[:], ones1[:], b2t[:], start=False, stop=True)
                comb = cpool.tile([128, 2 * CH], f32, tag=f"AB{s}")
                if s % 2 == 0:
                    nc.vector.tensor_copy(out=comb[:], in_=pa[:])
                else:
                    nc.scalar.copy(out=comb[:], in_=pa[:])
                A_t.append(comb[:, 0:CH])
                Bp_t.append(comb[:, CH:2 * CH])

            # ---- phase 1b: mixed column tables Cmix = R*B' + L*A (f32) ---
            Cmix = []
            for s in range(4):
                eng = nc.vector if s % 2 == 0 else nc.gpsimd
                t1 = tpool.tile([128, CH], f32, tag="t1")
                eng.tensor_scalar(t1[:], Bp_t[s], cmt[:, s:s + 1], None,
                                  mybir.AluOpType.mult)
                t2 = tpool.tile([128, CH], f32, tag="t2")
                eng.tensor_scalar(t2[:], A_t[s], cmt[:, 4 + s:5 + s], None,
                                  mybir.AluOpType.mult)
                cx = cpool.tile([128, CH], f32, tag=f"C{s}")
                eng.tensor_tensor(cx[:], t1[:], t2[:], mybir.AluOpType.add)
                Cmix.append(cx)

            # duplicated f32 column tables for the r-paired main tiles
            CD = {}
            for s in (1, 2, 3):
                dup = cpool.tile([128, 2 * CH], f32, tag=f"CD{s}")
                nc.vector.tensor_copy(out=dup[:, 0:CH], in_=Cmix[s][:])
                nc.scalar.copy(out=dup[:, CH:2 * CH], in_=Cmix[s][:])
                CD[s] = dup


            def hi_lo(src_ap, tag):
                """split a f32 [128, W] AP into bf16 hi + lo tiles."""
                wdt = src_ap.shape[-1]
                hi = cpool.tile([128, wdt], BF16, tag=f"{tag}h")
                nc.scalar.copy(out=hi[:], in_=src_ap)
                h32 = tpool.tile([128, wdt], f32, tag="h32")
                nc.scalar.copy(out=h32[:], in_=hi[:])
                d = tpool.tile([128, wdt], f32, tag="d32")
                nc.vector.tensor_sub(d[:], src_ap, h32[:])
                lo = cpool.tile([128, wdt], BF16, tag=f"{tag}l")
                nc.vector.tensor_copy(out=lo[:], in_=d[:])
                return hi, lo

            ah, al = hi_lo(A_t[0], "a0")
            bh, bl = hi_lo(Bp_t[0], "b0")
            # block0-upper f32 column table, replicated to both q-halves
            cup = cpool.tile([128, CH], f32, tag="cup")
            nc.gpsimd.dma_start(out=cup[0:64, :], in_=Cmix[0][64:128, :])
            nc.gpsimd.dma_start(out=cup[64:128, :], in_=Cmix[0][64:128, :])
            # diag combined rhs [A_rot[0:64] ; B'_rot[0:64]] (hi / lo)
            dcb_h = cpool.tile([128, CH], BF16, tag="dcbh")
            dcb_l = cpool.tile([128, CH], BF16, tag="dcbl")
            nc.vector.tensor_copy(out=dcb_h[0:64, :], in_=ah[0:64, :])
            nc.vector.tensor_copy(out=dcb_l[0:64, :], in_=al[0:64, :])
            nc.gpsimd.dma_start(out=dcb_h[64:128, :], in_=bh[0:64, :])
            nc.gpsimd.dma_start(out=dcb_l[64:128, :], in_=bl[0:64, :])
            # flat row tables: direct SBUF->SBUF flatten into partitions
            # 0-7, then replicated to partition groups 32/64/96 (walrus
            # requires rhs to start at the same partition as the weights).
            rp4 = cpool.tile([104, 64 * CH], BF16, tag="rp4")
            nc.vector.memset(rp4[0:8, 63 * CH:64 * CH], 0.0)
            for i, t in enumerate((ah, bh, al, bl)):
                nc.gpsimd.dma_start(out=rp4[i:i + 1, :], in_=t[0:64, :])
                nc.gpsimd.dma_start(out=rp4[4 + i:5 + i, 0:63 * CH],
                                  in_=t[1:64, :])
            for gp in (32, 64, 96):
                nc.gpsimd.dma_start(out=rp4[gp:gp + 8, :], in_=rp4[0:8, :])

            # ---- phase 2: main loop --------------------------------------
            for g in range(8):
                sM = {J: stM.tile([128, 4 * 512], f32, tag="sm",
                                  name=f"sm_{g}_{J}")
                      for J in (1, 2, 3)}
                s0d = st0.tile([128, 4 * CH], f32, tag="s0")
                s0u = st0.tile([128, 4 * CH], f32, tag="s0")
                for sub in range(4):
                    rp = 4 * g + sub
                    off = 2 * rp * CH
                    # four small-K masked-broadcast matmuls in distinct PE
                    # row-groups -> concurrent execution.
                    pj = {}
                    for J in (1, 2, 3):
                        gp = 32 * (J - 1)
                        p = psA.tile([128, 512], f32, tag="pj",
                                     name=f"pj_{rp}_{J}")
                        nc.tensor.matmul(
                            p[:], wmt[gp:gp + 4, 128 * J:128 * (J + 1)],
                            rp4[gp:gp + 4, off:off + 512],
                            start=True, stop=True, tile_position=(gp, 0))
                        pj[J] = p
                    pu = ps0.tile([128, CH], f32, tag="p0", name=f"pu_{rp}")
                    nc.tensor.matmul(
                        pu[:], wmt[96:104, 0:128],
                        rp4[96:104, off:off + CH],
                        start=True, stop=True, tile_position=(96, 0))
                    # diagonal block (s in [0,64)), rows r0, r0+1
                    pd = ps0.tile([128, CH], f32, tag="p0", name=f"pd_{rp}")
                    dl_sl = dlt[:, 128 * rp:128 * (rp + 1)]
                    dr_sl = drt[:, 128 * rp:128 * (rp + 1)]
                    nc.tensor.matmul(pd[:], dl_sl, dcb_h[:],
                                     start=True, stop=False)
                    nc.tensor.matmul(pd[:], dl_sl, dcb_l[:],
                                     start=False, stop=False)
                    nc.tensor.matmul(pd[:], dr_sl, dcb_h[:],
                                     start=False, stop=False)
                    nc.tensor.matmul(pd[:], dr_sl, dcb_l[:],
                                     start=False, stop=True)
                    # evacuation: VectorE folds the f32 column tables in;
                    # ScalarE evacuates the diagonal block.
                    for J in (1, 2, 3):
                        nc.vector.tensor_add(
                            sM[J][:, 512 * sub:512 * (sub + 1)],
                            pj[J][:], CD[J][:])
                    nc.vector.tensor_add(
                        s0u[:, CH * sub:CH * (sub + 1)], pu[:], cup[:])
                    nc.scalar.copy(out=s0d[:, CH * sub:CH * (sub + 1)],
                                   in_=pd[:])
                    if sub in (1, 3):
                        h = (sub - 1) // 2
                        hs, he = 1024 * h, 1024 * (h + 1)
                        for J in (1, 2):
                            nc.sync.dma_start(
                                out=slab_m[3 * g + J - 1][:, hs:he],
                                in_=sM[J][:, hs:he])
                        nc.scalar.dma_start(
                            out=slab_m[3 * g + 2][:, hs:he],
                            in_=sM[3][:, hs:he])
                        nc.scalar.dma_start(
                            out=out0u[g][:, 512 * h:512 * (h + 1)],
                            in_=s0u[:, 512 * h:512 * (h + 1)])
                        nc.scalar.dma_start(
                            out=out0d[g][:, 512 * h:512 * (h + 1)],
                            in_=s0d[:, 512 * h:512 * (h + 1)])

    nc.compile()
    return nc


def _program() -> bass.Bass:
    global _PROGRAM
    if _PROGRAM is None:
        _PROGRAM = _build_program()
    return _PROGRAM


# --------------------------------------------------------------------------
# host entry point
# --------------------------------------------------------------------------

def _assemble(results):
    """8 per-core result dicts -> full [512, 512, 256] output."""
    out = np.empty((N, N, CH), np.float32)
    for k in range(NCORES):
        r = results[k]
        slab = np.empty((RB, N, CH), np.float32)
        # out0d/u: [g, q*64+s, (sub, ch)] -> rows 8g+2sub+q, cols s / 64+s
        d = np.asarray(r["out0d"]).reshape(8, 2, 64, 4, CH)
        slab[:, 0:64, :] = d.transpose(0, 3, 1, 2, 4).reshape(RB, 64, CH)
        u = np.asarray(r["out0u"]).reshape(8, 2, 64, 4, CH)
        slab[:, 64:128, :] = u.transpose(0, 3, 1, 2, 4).reshape(RB, 64, CH)
        # slab_m: [3g+J-1, p, (sub, q, ch)] -> rows 8g+2sub+q, col 128J+p
        m = np.asarray(r["slab_m"]).reshape(8, 3, 128, 4, 2, CH)
        slab[:, 128:512, :] = (
            m.transpose(0, 3, 4, 1, 2, 5).reshape(RB, 384, CH))
        base = RB * k
        out[base:base + RB] = np.roll(slab, base, axis=1)
    return out


def build_in_maps(x, W, b):
    shared = _shared_inputs(W, b)
    return [dict(shared, **_core_inputs(x, k)) for k in range(NCORES)]


def kernel(x, W, b):
    nc = _program()
    in_maps = build_in_maps(x, W, b)
    res = run_bass_kernel_spmd(nc, in_maps, core_ids=list(range(NCORES)))
    return _assemble(res.results)



# revision 11
# speedup vs baseline: 1.6191x; 1.6191x over previous
"""Trainium2 Bass kernel for nn_LinearEncoder (gnn_message_passing).

Reference computes, for N=512 nodes with n_in = n_out = 256:
    i, j = triu_indices(N, k=1)
    edges = concat([x[i], x[j]], -1)            # [E, 512]
    h = edges @ W.T + b                         # [E, n_out]
    out[i, j] = h ; out = out + out.T           # [N, N, 256], 0 diagonal

Key algebraic identity: with W = [W1 | W2],
    h(i, j) = A[i] + B[j] + b,   A = x @ W1.T,  B = x @ W2.T
so the full output is
    out[i, j] = A[min(i,j)] + B'[max(i,j)]      (B' = B + b), 0 on diagonal.

Sharding: output rows split across 8 cores (64 rows each), one SPMD
program.  Core k receives x pre-rotated by its row base
(x_rot[t] = x[(base+t) % 512]) so the triangular "diagonal block" sits at
local columns s in [0, 64) on every core; region selection (A vs B')
enters only through small 0/1 mask *inputs*.

v3: end-to-end bf16 datapath (host pre-casts inputs, bf16 output slabs
re-cast to f32 on host — well inside the tolerance), halving the HBM
write roofline.  Per row-pair rp (rows r0 = 2rp, r0+1):
  - four concurrent small-K masked-broadcast matmuls in distinct PE
    row-groups produce the row terms of the three main 128-wide column
    blocks and (via shifted flat tables, K=4) the block0-upper halves;
  - two full-array matmuls follow: an identity matmul folding J3's bf16
    column table into PSUM, and the combined L+R constant selection for
    the triangular diagonal block (exact-zero diagonal);
  - evacuation is balanced: VectorE adds the column tables into J1|J2
    (one 2-bank op) and block0-upper, ScalarE copies out the
    PE-complete J3 and diagonal tiles.
  - per group of 4 row-pairs everything lands in one [128, 8192] bf16
    tile shipped as two 1 MB HWDGE DMAs (16 total per core).
"""

import os
import sys

for _p in ("/opt/trn_rl_repo", "/root/.axon_site/_ro/trn_rl_repo"):
    if os.path.isdir(_p) and _p not in sys.path:
        sys.path.insert(0, _p)

import numpy as np
import ml_dtypes

import concourse.bass as bass
import concourse.bacc as bacc
import concourse.mybir as mybir
import concourse.tile as tile
from concourse.bass_utils import run_bass_kernel_spmd

N = 512
CH = 256          # n_out
NIN = 256         # n_in
NCORES = 8
RB = N // NCORES  # 64 rows per core
F32 = mybir.dt.float32
BF16 = mybir.dt.bfloat16
BF16NP = ml_dtypes.bfloat16


# --------------------------------------------------------------------------
# host-side constant builders
# --------------------------------------------------------------------------

def _masks_RL(k: int):
    """R/L region indicators over local columns s for core k."""
    base = RB * k
    wrap = N - base  # columns s >= wrap hold wrapped (j < base) entries
    s = np.arange(N)
    R = ((s >= 64) & (s < wrap)).astype(np.float32)
    L = (s >= wrap).astype(np.float32)
    return R, L


def _diag_const():
    """Combined masked-selection weights for the 64x64 diagonal blocks.

    For row-pair rp, output column m = q*64 + s (q in {0,1}, s in [0,64)),
    with r_q = 2*rp + q and rhs dcb = [A_rot[0:64] ; B'_rot[0:64]] (K = 128):
      s < r_q:  value = A_rot[s] + B'_rot[r_q]
      s > r_q:  value = B'_rot[s] + A_rot[r_q]
      s == r_q: all weights zero -> exact 0 output.
    """
    d = np.zeros((128, 32 * 128), np.float32)
    for rp in range(32):
        for q in range(2):
            r_q = 2 * rp + q
            for s in range(64):
                m = rp * 128 + q * 64 + s
                if s < r_q:
                    d[s, m] = 1.0            # A_rot[s]
                    d[64 + r_q, m] = 1.0     # B'_rot[r_q]
                elif s > r_q:
                    d[64 + s, m] = 1.0       # B'_rot[s]
                    d[r_q, m] = 1.0          # A_rot[r_q]
    return d


def _shared_inputs(W: np.ndarray, b: np.ndarray):
    W = np.asarray(W, np.float32)
    b = np.asarray(b, np.float32)
    w12 = np.concatenate(
        [np.ascontiguousarray(W[:, :NIN].T), np.ascontiguousarray(W[:, NIN:].T)],
        axis=1)                                     # [in, 2*out] = [A | B]
    b2bc = np.broadcast_to(
        np.concatenate([np.zeros(CH, np.float32), b]), (128, 2 * CH))
    ident = np.eye(128, dtype=np.float32)
    return {
        "w12t": w12.astype(BF16NP),
        "b2bc": np.ascontiguousarray(b2bc).astype(BF16NP),
        "dcomb": _diag_const().astype(BF16NP),
        "identw": ident.astype(BF16NP),
    }


def _core_inputs(x: np.ndarray, k: int):
    x = np.asarray(x, np.float32)
    base = RB * k
    x_rot = np.roll(x, -base, axis=0)
    R, L = _masks_RL(k)

    # column-table masks: cm[:, s] = R over node block s, cm[:, 4+s] = L
    cm = np.zeros((128, 8), np.float32)
    for t in range(4):
        cm[:, t] = R[128 * t:128 * (t + 1)]
        cm[:, 4 + t] = L[128 * t:128 * (t + 1)]

    # wmt [128, 512]: per-row-group small-K masked-broadcast weights.
    #   rows 0-1   cols 128:256 -> J=1   (w0 = R, w1 = L over that block)
    #   rows 32-33 cols 256:384 -> J=2
    #   rows 64-65 cols 384:512 -> J=3
    #   rows 96-99 cols 0:128   -> block0-upper K=4 (rows 96-97 hit the
    #     plain flats for row r0 at p<64, rows 98-99 the +256-shifted
    #     flats for row r0+1 at p>=64)
    wm = np.zeros((128, 512), np.float32)
    for J in (1, 2, 3):
        gp = 32 * (J - 1)
        wm[gp, 128 * J:128 * (J + 1)] = R[128 * J:128 * (J + 1)]
        wm[gp + 1, 128 * J:128 * (J + 1)] = L[128 * J:128 * (J + 1)]
    p = np.arange(64)
    wm[96, p] = R[64 + p]
    wm[97, p] = L[64 + p]
    wm[98, 64 + p] = R[64 + p]
    wm[99, 64 + p] = L[64 + p]
    return {
        "xt_rot": np.ascontiguousarray(x_rot.T).astype(BF16NP),  # [in, node]
        "cm": cm,
        "wm": wm.astype(BF16NP),
    }


# --------------------------------------------------------------------------
# device program
# --------------------------------------------------------------------------

_PROGRAM = None


def _build_program() -> bass.Bass:
    nc = bacc.Bacc()
    f32 = F32

    # ---- dram tensors -----------------------------------------------------
    xt_rot = nc.dram_tensor("xt_rot", [NIN, N], BF16, kind="ExternalInput")
    w12t = nc.dram_tensor("w12t", [NIN, 2 * CH], BF16, kind="ExternalInput")
    b2bc = nc.dram_tensor("b2bc", [128, 2 * CH], BF16, kind="ExternalInput")
    cm = nc.dram_tensor("cm", [128, 8], F32, kind="ExternalInput")
    d_wm = nc.dram_tensor("wm", [128, N], BF16, kind="ExternalInput")
    d_dc = nc.dram_tensor("dcomb", [128, 32 * 128], BF16, kind="ExternalInput")
    d_id = nc.dram_tensor("identw", [128, 128], BF16, kind="ExternalInput")

    # slab[g, p, :]: cols 1024*sub+512*Jh+256*q+c = (row 8g+2sub+q, col
    # 128(Jh+1)+p) for Jh in {0,1}; cols 4096+512*sub+256*q+c = J3;
    # cols 6144+256*sub -> diag (p = q*64+s); cols 7168+256*sub -> upper
    # (p<64: row r0, s=64+p ; p>=64: row r0+1, s=p).  Host unpicks.
    slab = nc.dram_tensor("slab", [8, 128, 8192], BF16, kind="ExternalOutput")

    with tile.TileContext(nc) as tc:
        with (
            tc.tile_pool(name="const", bufs=1) as cpool,
            tc.tile_pool(name="tmp", bufs=2) as tpool,
            tc.tile_pool(name="psJ", bufs=2, space="PSUM") as psJ,
            tc.tile_pool(name="ps3", bufs=2, space="PSUM") as ps3,
            tc.tile_pool(name="psDU", bufs=2, space="PSUM") as psDU,
            tc.tile_pool(name="stS", bufs=2) as stS,
        ):
            # ---- load inputs (spread across HWDGE queues) ----------------
            def load(eng, dram, shape, dtype, tag):
                t = cpool.tile(shape, dtype, tag=tag)
                eng.dma_start(out=t[:], in_=dram[:])
                return t

            xt0 = load(nc.sync, xt_rot[0:128, :], [128, N], BF16, "xt0")
            w12a = load(nc.scalar, w12t[0:128, :], [128, 2 * CH], BF16, "w12a")
            xt1 = load(nc.sync, xt_rot[128:256, :], [128, N], BF16, "xt1")
            w12b = load(nc.scalar, w12t[128:256, :], [128, 2 * CH], BF16,
                        "w12b")
            b2t = load(nc.gpsimd, b2bc, [128, 2 * CH], BF16, "b2t")
            cmt = load(nc.gpsimd, cm, [128, 8], F32, "cmt")
            wmt = load(nc.gpsimd, d_wm, [128, N], BF16, "wmt")
            idt = load(nc.gpsimd, d_id, [128, 128], BF16, "idt")
            dct = load(nc.scalar, d_dc, [128, 32 * 128], BF16, "dct")

            # ---- phase 1: tables [A | B'] (bf16), one psum per s-block ---
            # evac folds the (pre-broadcast) bias row in: A half is a pure
            # ScalarE cast, B half a VectorE add of b2bc.
            AB = []
            for s in range(4):
                pa = ps3.tile([128, 2 * CH], f32, tag="p3", name=f"ptb{s}")
                mmd = nc.tensor.matmul
                mmd(pa[:], xt0[:, 128 * s:128 * (s + 1)], w12a[:],
                    start=True, stop=False)
                mmd(pa[:], xt1[:, 128 * s:128 * (s + 1)], w12b[:],
                    start=False, stop=True)
                comb = cpool.tile([128, 2 * CH], BF16, tag=f"AB{s}")
                nc.scalar.copy(out=comb[:, 0:CH], in_=pa[:, 0:CH])
                nc.vector.tensor_add(comb[:, CH:2 * CH], pa[:, CH:2 * CH],
                                     b2t[:, CH:2 * CH])
                AB.append(comb)

            # ---- mixed column tables Cmix_s = R*B' + L*A (bf16) ----------
            Cmix = []
            for s in range(4):
                t1 = tpool.tile([128, CH], BF16, tag="t1")
                nc.vector.tensor_scalar(t1[:], AB[s][:, 0:CH],
                                        cmt[:, 4 + s:5 + s], None,
                                        mybir.AluOpType.mult)
                cx = cpool.tile([128, CH], BF16, tag=f"C{s}")
                nc.vector.scalar_tensor_tensor(
                    cx[:], AB[s][:, CH:2 * CH], cmt[:, s:s + 1], t1[:],
                    mybir.AluOpType.mult, mybir.AluOpType.add)
                Cmix.append(cx)

            # duplicated column tables: CD12 = [C1|C1|C2|C2], CD3 = [C3|C3]
            CD12 = cpool.tile([128, 4 * CH], BF16, tag="CD12")
            CD3 = cpool.tile([128, 2 * CH], BF16, tag="CD3")
            for h in range(2):
                nc.gpsimd.tensor_copy(out=CD12[:, CH * h:CH * (h + 1)],
                                      in_=Cmix[1][:])
                nc.gpsimd.tensor_copy(out=CD12[:, CH * (2 + h):CH * (3 + h)],
                                      in_=Cmix[2][:])
                nc.gpsimd.tensor_copy(out=CD3[:, CH * h:CH * (h + 1)],
                                      in_=Cmix[3][:])
            # block0-upper column table, replicated to both q-halves
            cup = cpool.tile([128, CH], BF16, tag="cup")
            nc.scalar.dma_start(out=cup[0:64, :], in_=Cmix[0][64:128, :])
            nc.vector.tensor_copy(out=cup[64:128, :], in_=Cmix[0][64:128, :])

            # diag combined rhs dcb = [A_rot[0:64] ; B'_rot[0:64]]
            dcb = cpool.tile([128, CH], BF16, tag="dcb")
            nc.vector.tensor_copy(out=dcb[0:64, :], in_=AB[0][0:64, 0:CH])
            nc.scalar.dma_start(out=dcb[64:128, :], in_=AB[0][0:64, CH:2 * CH])

            # flat row tables ft: row gp = A nodes 0..63 flattened, gp+1 =
            # B'; rows 98/99 are the +1-node-shifted flats for block0-upper
            # row r0+1.  Independent direct builds, spread across queues.
            ft = cpool.tile([128, 64 * CH], BF16, tag="ft")
            for i, gp in enumerate((0, 32, 64, 96)):
                qa = (nc.sync, nc.scalar, nc.gpsimd)[i % 3]
                qb = (nc.scalar, nc.gpsimd, nc.sync)[i % 3]
                qa.dma_start(out=ft[gp:gp + 1, :], in_=AB[0][0:64, 0:CH])
                qb.dma_start(out=ft[gp + 1:gp + 2, :],
                             in_=AB[0][0:64, CH:2 * CH])
            nc.sync.dma_start(out=ft[98:99, 0:63 * CH],
                              in_=AB[0][1:64, 0:CH])
            nc.gpsimd.dma_start(out=ft[99:100, 0:63 * CH],
                                in_=AB[0][1:64, CH:2 * CH])

            # ---- phase 2: main loop --------------------------------------
            for g in range(8):
                S = stS.tile([128, 8192], BF16, tag="s", name=f"s_{g}")
                for sub in range(4):
                    rp = 4 * g + sub
                    off = 2 * rp * CH
                    mm = nc.tensor.matmul
                    # four concurrent row-group matmuls (row terms)
                    pj = psJ.tile([128, 1024], f32, tag="pj",
                                  name=f"pj_{rp}")
                    mm(pj[:, 0:512], wmt[0:2, 128:256],
                       ft[0:2, off:off + 512],
                       start=True, stop=True, tile_position=(0, 0))
                    mm(pj[:, 512:1024], wmt[32:34, 256:384],
                       ft[32:34, off:off + 512],
                       start=True, stop=True, tile_position=(32, 0))
                    p3 = ps3.tile([128, 512], f32, tag="p3", name=f"p3_{rp}")
                    mm(p3[:], wmt[64:66, 384:512], ft[64:66, off:off + 512],
                       start=True, stop=False, tile_position=(64, 0))
                    pdu = psDU.tile([128, 512], f32, tag="pdu",
                                    name=f"pdu_{rp}")
                    mm(pdu[:, 256:512], wmt[96:100, 0:128],
                       ft[96:100, off:off + 256],
                       start=True, stop=True, tile_position=(96, 0))
                    # full-array matmuls: J3 column fold + diag selection
                    mm(p3[:], idt[:], CD3[:],
                       start=False, stop=True, skip_group_check=True)
                    mm(pdu[:, 0:256], dct[:, 128 * rp:128 * (rp + 1)],
                       dcb[:], start=True, stop=True)
                    # evacuation: DVE adds column tables into J1|J2 and
                    # upper; ScalarE copies the PE-complete J3 and diag.
                    nc.vector.tensor_add(
                        S[:, 1024 * sub:1024 * (sub + 1)], pj[:], CD12[:])
                    nc.scalar.copy(
                        out=S[:, 4096 + 512 * sub:4096 + 512 * (sub + 1)],
                        in_=p3[:])
                    nc.vector.tensor_add(
                        S[:, 7168 + 256 * sub:7168 + 256 * (sub + 1)],
                        pdu[:, 256:512], cup[:])
                    nc.scalar.copy(
                        out=S[:, 6144 + 256 * sub:6144 + 256 * (sub + 1)],
                        in_=pdu[:, 0:256])
                    if sub == 1:
                        nc.sync.dma_start(out=slab[g][:, 0:2048],
                                          in_=S[:, 0:2048])
                    elif sub == 3:
                        nc.sync.dma_start(out=slab[g][:, 2048:8192],
                                          in_=S[:, 2048:8192])

    nc.compile()
    return nc


def _program() -> bass.Bass:
    global _PROGRAM
    if _PROGRAM is None:
        _PROGRAM = _build_program()
    return _PROGRAM


# --------------------------------------------------------------------------
# host entry point
# --------------------------------------------------------------------------

def _assemble(results):
    """8 per-core result dicts -> full [512, 512, 256] output."""
    out = np.empty((N, N, CH), np.float32)
    for k in range(NCORES):
        r = results[k]
        m = np.asarray(r["slab"]).reshape(8, 128, 8192).astype(np.float32)
        slab = np.empty((RB, N, CH), np.float32)
        # main J blocks: [g, p, 1024*sub + 512*Jh + 256*q + c]
        j12 = m[:, :, 0:4096].reshape(8, 128, 4, 2, 2, CH)
        slab[:, 128:384, :] = (
            j12.transpose(0, 2, 4, 3, 1, 5)       # g, sub, q, Jh, p, c
            .reshape(RB, 2 * 128, CH))
        j3 = m[:, :, 4096:6144].reshape(8, 128, 4, 2, CH)
        slab[:, 384:512, :] = (
            j3.transpose(0, 2, 3, 1, 4).reshape(RB, 128, CH))
        # diag: [g, q*64+s, 256*sub + c] -> rows 8g+2sub+q, col s
        dg = m[:, :, 6144:7168].reshape(8, 2, 64, 4, CH)
        slab[:, 0:64, :] = dg.transpose(0, 3, 1, 2, 4).reshape(RB, 64, CH)
        # upper: p<64 -> (q=0, s=64+p); p>=64 -> (q=1, s=p)
        up = m[:, :, 7168:8192].reshape(8, 2, 64, 4, CH)
        slab[:, 64:128, :] = up.transpose(0, 3, 1, 2, 4).reshape(RB, 64, CH)
        base = RB * k
        out[base:base + RB] = np.roll(slab, base, axis=1)
    return out


def build_in_maps(x, W, b):
    shared = _shared_inputs(W, b)
    return [dict(shared, **_core_inputs(x, k)) for k in range(NCORES)]


def kernel(x, W, b):
    nc = _program()
    in_maps = build_in_maps(x, W, b)
    res = run_bass_kernel_spmd(nc, in_maps, core_ids=list(range(NCORES)))
    return _assemble(res.results)


# revision 18
# speedup vs baseline: 1.8265x; 1.1281x over previous
"""Trainium2 Bass kernel for nn_LinearEncoder (gnn_message_passing).

Reference computes, for N=512 nodes with n_in = n_out = 256:
    i, j = triu_indices(N, k=1)
    edges = concat([x[i], x[j]], -1)            # [E, 512]
    h = edges @ W.T + b                         # [E, n_out]
    out[i, j] = h ; out = out + out.T           # [N, N, 256], 0 diagonal

Key algebraic identity: with W = [W1 | W2],
    h(i, j) = A[i] + B[j] + b,   A = x @ W1.T,  B = x @ W2.T
so the full output is
    out[i, j] = A[min(i,j)] + B'[max(i,j)]      (B' = B + b), 0 on diagonal.

Sharding: output rows split across 8 cores (64 rows each), one SPMD
program.  Core k receives x pre-rotated by its row base
(x_rot[t] = x[(base+t) % 512]) so the triangular "diagonal block" sits at
local columns s in [0, 64) on every core; region selection (A vs B')
enters only through small 0/1 mask *inputs*.

v3: end-to-end bf16 datapath (host pre-casts inputs, bf16 output slabs
re-cast to f32 on host — well inside the tolerance), halving the HBM
write roofline.  Per row-pair rp (rows r0 = 2rp, r0+1):
  - four concurrent small-K masked-broadcast matmuls in distinct PE
    row-groups produce the row terms of the three main 128-wide column
    blocks and (via shifted flat tables, K=4) the block0-upper halves;
  - two full-array matmuls follow: an identity matmul folding J3's bf16
    column table into PSUM, and the combined L+R constant selection for
    the triangular diagonal block (exact-zero diagonal);
  - evacuation is balanced: VectorE adds the column tables into J1|J2
    (one 2-bank op) and block0-upper, ScalarE copies out the
    PE-complete J3 and diagonal tiles.
  - per group of 4 row-pairs everything lands in one [128, 8192] bf16
    tile shipped as two 1 MB HWDGE DMAs (16 total per core).
"""

import os
import sys

for _p in ("/opt/trn_rl_repo", "/root/.axon_site/_ro/trn_rl_repo"):
    if os.path.isdir(_p) and _p not in sys.path:
        sys.path.insert(0, _p)

import numpy as np
import ml_dtypes

import concourse.bass as bass
import concourse.bacc as bacc
import concourse.mybir as mybir
import concourse.tile as tile
from concourse.bass_utils import run_bass_kernel_spmd

N = 512
CH = 256          # n_out
NIN = 256         # n_in
NCORES = 8
RB = N // NCORES  # 64 rows per core
F32 = mybir.dt.float32
BF16 = mybir.dt.bfloat16
BF16NP = ml_dtypes.bfloat16


# --------------------------------------------------------------------------
# host-side constant builders
# --------------------------------------------------------------------------

def _masks_RL(k: int):
    """R/L region indicators over local columns s for core k."""
    base = RB * k
    wrap = N - base  # columns s >= wrap hold wrapped (j < base) entries
    s = np.arange(N)
    R = ((s >= 64) & (s < wrap)).astype(np.float32)
    L = (s >= wrap).astype(np.float32)
    return R, L


def _diag_const():
    """Combined masked-selection weights for the 64x64 diagonal blocks.

    For row-pair rp, output column m = q*64 + s (q in {0,1}, s in [0,64)),
    with r_q = 2*rp + q and rhs dcb = [A_rot[0:64] ; B'_rot[0:64]] (K = 128):
      s < r_q:  value = A_rot[s] + B'_rot[r_q]
      s > r_q:  value = B'_rot[s] + A_rot[r_q]
      s == r_q: all weights zero -> exact 0 output.
    """
    d = np.zeros((128, 32 * 128), np.float32)
    for rp in range(32):
        for q in range(2):
            r_q = 2 * rp + q
            for s in range(64):
                m = rp * 128 + q * 64 + s
                if s < r_q:
                    d[s, m] = 1.0            # A_rot[s]
                    d[64 + r_q, m] = 1.0     # B'_rot[r_q]
                elif s > r_q:
                    d[64 + s, m] = 1.0       # B'_rot[s]
                    d[r_q, m] = 1.0          # A_rot[r_q]
    return d


def _shared_inputs(W: np.ndarray, b: np.ndarray):
    W = np.asarray(W, np.float32)
    b = np.asarray(b, np.float32)
    w12 = np.concatenate(
        [np.ascontiguousarray(W[:, :NIN].T), np.ascontiguousarray(W[:, NIN:].T)],
        axis=1)                                     # [in, 2*out] = [A | B]
    b2bc = np.broadcast_to(
        np.concatenate([np.zeros(CH, np.float32), b]), (128, 2 * CH))
    ident = np.eye(128, dtype=np.float32)
    return {
        "w12t": w12.astype(BF16NP),
        "b2bc": np.ascontiguousarray(b2bc).astype(BF16NP),
        "dcomb": _diag_const().astype(BF16NP),
        "identw": ident.astype(BF16NP),
    }


def _core_inputs(x: np.ndarray, k: int):
    x = np.asarray(x, np.float32)
    base = RB * k
    x_rot = np.roll(x, -base, axis=0)
    R, L = _masks_RL(k)

    # column-table masks: cm[:, s] = R over node block s, cm[:, 4+s] = L
    cm = np.zeros((128, 8), np.float32)
    for t in range(4):
        cm[:, t] = R[128 * t:128 * (t + 1)]
        cm[:, 4 + t] = L[128 * t:128 * (t + 1)]

    # wmt [128, 512]: per-row-group small-K masked-broadcast weights.
    #   rows 0-1   cols 128:256 -> J=1   (w0 = R, w1 = L over that block)
    #   rows 32-33 cols 256:384 -> J=2
    #   rows 64-65 cols 384:512 -> J=3
    #   rows 96-99 cols 0:128   -> block0-upper K=4 (rows 96-97 hit the
    #     plain flats for row r0 at p<64, rows 98-99 the +256-shifted
    #     flats for row r0+1 at p>=64)
    wm = np.zeros((128, 512), np.float32)
    for J in (1, 2, 3):
        gp = 32 * (J - 1)
        wm[gp, 128 * J:128 * (J + 1)] = R[128 * J:128 * (J + 1)]
        wm[gp + 1, 128 * J:128 * (J + 1)] = L[128 * J:128 * (J + 1)]
    p = np.arange(64)
    wm[96, p] = R[64 + p]
    wm[97, p] = L[64 + p]
    wm[98, 64 + p] = R[64 + p]
    wm[99, 64 + p] = L[64 + p]
    return {
        "xt_rot": np.ascontiguousarray(x_rot.T).astype(BF16NP),  # [in, node]
        "cm": cm,
        "wm": wm.astype(BF16NP),
    }


# --------------------------------------------------------------------------
# device program
# --------------------------------------------------------------------------

_PROGRAM = None


def _build_program() -> bass.Bass:
    nc = bacc.Bacc()
    f32 = F32

    # ---- dram tensors -----------------------------------------------------
    xt_rot = nc.dram_tensor("xt_rot", [NIN, N], BF16, kind="ExternalInput")
    w12t = nc.dram_tensor("w12t", [NIN, 2 * CH], BF16, kind="ExternalInput")
    b2bc = nc.dram_tensor("b2bc", [128, 2 * CH], BF16, kind="ExternalInput")
    cm = nc.dram_tensor("cm", [128, 8], F32, kind="ExternalInput")
    d_wm = nc.dram_tensor("wm", [128, N], BF16, kind="ExternalInput")
    d_dc = nc.dram_tensor("dcomb", [128, 32 * 128], BF16, kind="ExternalInput")
    d_id = nc.dram_tensor("identw", [128, 128], BF16, kind="ExternalInput")

    # slab[g, p, :]: two 4096-col halves h = sub//2 (u = sub%2), each the
    # DMA unit: base = 4096h; J12 at base+1024u (512*Jh+256*q+c ->
    # row 8g+2sub+q, col 128(Jh+1)+p); J3 at base+2048+512u; diag at
    # base+3072+256u (p = q*64+s); upper at base+3584+256u (p<64: row r0,
    # s=64+p ; p>=64: row r0+1, s=p).  Host unpicks.
    slab = nc.dram_tensor("slab", [8, 128, 8192], BF16, kind="ExternalOutput")

    with tile.TileContext(nc) as tc:
        with (
            tc.tile_pool(name="const", bufs=1) as cpool,
            tc.tile_pool(name="tmp", bufs=2) as tpool,
            tc.tile_pool(name="psJ", bufs=2, space="PSUM") as psJ,
            tc.tile_pool(name="ps3", bufs=2, space="PSUM") as ps3,
            tc.tile_pool(name="psDU", bufs=2, space="PSUM") as psDU,
            tc.tile_pool(name="stS", bufs=2) as stS,
        ):
            # ---- load inputs (spread across HWDGE queues) ----------------
            def load(eng, dram, shape, dtype, tag):
                t = cpool.tile(shape, dtype, tag=tag)
                eng.dma_start(out=t[:], in_=dram[:])
                return t

            xt0 = load(nc.sync, xt_rot[0:128, :], [128, N], BF16, "xt0")
            w12a = load(nc.scalar, w12t[0:128, :], [128, 2 * CH], BF16, "w12a")
            xt1 = load(nc.sync, xt_rot[128:256, :], [128, N], BF16, "xt1")
            w12b = load(nc.scalar, w12t[128:256, :], [128, 2 * CH], BF16,
                        "w12b")
            b2t = load(nc.gpsimd, b2bc, [128, 2 * CH], BF16, "b2t")
            cmt = load(nc.gpsimd, cm, [128, 8], F32, "cmt")
            wmt = load(nc.gpsimd, d_wm, [128, N], BF16, "wmt")
            idt = load(nc.gpsimd, d_id, [128, 128], BF16, "idt")
            dct = load(nc.scalar, d_dc, [128, 32 * 128], BF16, "dct")

            # ---- phase 1: tables [A | B'] (bf16), one psum per s-block ---
            # evac folds the (pre-broadcast) bias row in: A half is a pure
            # ScalarE cast, B half a VectorE add of b2bc.
            AB = []
            for s in range(4):
                pa = ps3.tile([128, 2 * CH], f32, tag="p3", name=f"ptb{s}")
                mmd = nc.tensor.matmul
                mmd(pa[:], xt0[:, 128 * s:128 * (s + 1)], w12a[:],
                    start=True, stop=False)
                mmd(pa[:], xt1[:, 128 * s:128 * (s + 1)], w12b[:],
                    start=False, stop=True)
                comb = cpool.tile([128, 2 * CH], BF16, tag=f"AB{s}")
                nc.scalar.copy(out=comb[:, 0:CH], in_=pa[:, 0:CH])
                nc.vector.tensor_add(comb[:, CH:2 * CH], pa[:, CH:2 * CH],
                                     b2t[:, CH:2 * CH])
                AB.append(comb)

            # ---- mixed column tables Cmix_s = R*B' + L*A (bf16) ----------
            Cmix = []
            for s in range(4):
                t1 = tpool.tile([128, CH], BF16, tag="t1")
                nc.vector.tensor_scalar(t1[:], AB[s][:, 0:CH],
                                        cmt[:, 4 + s:5 + s], None,
                                        mybir.AluOpType.mult)
                cx = cpool.tile([128, CH], BF16, tag=f"C{s}")
                nc.vector.scalar_tensor_tensor(
                    cx[:], AB[s][:, CH:2 * CH], cmt[:, s:s + 1], t1[:],
                    mybir.AluOpType.mult, mybir.AluOpType.add)
                Cmix.append(cx)

            # duplicated column tables: CD12 = [C1|C1|C2|C2], CD3 = [C3|C3]
            CD12 = cpool.tile([128, 4 * CH], BF16, tag="CD12")
            CD3 = cpool.tile([128, 2 * CH], BF16, tag="CD3")
            for h in range(2):
                nc.gpsimd.tensor_copy(out=CD12[:, CH * h:CH * (h + 1)],
                                      in_=Cmix[1][:])
                nc.gpsimd.tensor_copy(out=CD12[:, CH * (2 + h):CH * (3 + h)],
                                      in_=Cmix[2][:])
                nc.gpsimd.tensor_copy(out=CD3[:, CH * h:CH * (h + 1)],
                                      in_=Cmix[3][:])
            # block0-upper column table, replicated to both q-halves
            cup = cpool.tile([128, CH], BF16, tag="cup")
            nc.scalar.dma_start(out=cup[0:64, :], in_=Cmix[0][64:128, :])
            nc.vector.tensor_copy(out=cup[64:128, :], in_=Cmix[0][64:128, :])

            # diag combined rhs dcb = [A_rot[0:64] ; B'_rot[0:64]]
            dcb = cpool.tile([128, CH], BF16, tag="dcb")
            nc.vector.tensor_copy(out=dcb[0:64, :], in_=AB[0][0:64, 0:CH])
            nc.scalar.dma_start(out=dcb[64:128, :], in_=AB[0][0:64, CH:2 * CH])

            # flat row tables ft: row gp = A nodes 0..63 flattened, gp+1 =
            # B'; rows 98/99 are the +1-node-shifted flats for block0-upper
            # row r0+1.  Independent direct builds, spread across queues.
            ft = cpool.tile([128, 64 * CH], BF16, tag="ft")
            for i, gp in enumerate((0, 32, 64, 96)):
                qa = (nc.sync, nc.scalar, nc.gpsimd)[i % 3]
                qb = (nc.scalar, nc.gpsimd, nc.sync)[i % 3]
                qa.dma_start(out=ft[gp:gp + 1, :], in_=AB[0][0:64, 0:CH])
                qb.dma_start(out=ft[gp + 1:gp + 2, :],
                             in_=AB[0][0:64, CH:2 * CH])
            nc.sync.dma_start(out=ft[98:99, 0:63 * CH],
                              in_=AB[0][1:64, 0:CH])
            nc.gpsimd.dma_start(out=ft[99:100, 0:63 * CH],
                                in_=AB[0][1:64, CH:2 * CH])

            # ---- phase 2: main loop --------------------------------------
            for g in range(8):
                S = stS.tile([128, 8192], BF16, tag="s", name=f"s_{g}")
                for sub in range(4):
                    rp = 4 * g + sub
                    off = 2 * rp * CH
                    base = 4096 * (sub // 2)
                    u = sub % 2
                    mm = nc.tensor.matmul
                    # four concurrent row-group matmuls (row terms)
                    pj = psJ.tile([128, 1024], f32, tag="pj",
                                  name=f"pj_{rp}")
                    mm(pj[:, 0:512], wmt[0:2, 128:256],
                       ft[0:2, off:off + 512],
                       start=True, stop=True, tile_position=(0, 0))
                    mm(pj[:, 512:1024], wmt[32:34, 256:384],
                       ft[32:34, off:off + 512],
                       start=True, stop=True, tile_position=(32, 0))
                    p3 = ps3.tile([128, 512], f32, tag="p3", name=f"p3_{rp}")
                    mm(p3[:], wmt[64:66, 384:512], ft[64:66, off:off + 512],
                       start=True, stop=False, tile_position=(64, 0))
                    pdu = psDU.tile([128, 512], f32, tag="pdu",
                                    name=f"pdu_{rp}")
                    mm(pdu[:, 256:512], wmt[96:100, 0:128],
                       ft[96:100, off:off + 256],
                       start=True, stop=True, tile_position=(96, 0))
                    # full-array matmuls: J3 column fold + diag selection
                    mm(p3[:], idt[:], CD3[:],
                       start=False, stop=True, skip_group_check=True)
                    mm(pdu[:, 0:256], dct[:, 128 * rp:128 * (rp + 1)],
                       dcb[:], start=True, stop=True)
                    # evacuation: DVE adds column tables into upper (small,
                    # first, frees psU early) and J1|J2; ScalarE copies the
                    # PE-complete J3 and diag tiles.
                    nc.vector.tensor_add(
                        S[:, base + 3584 + 256 * u:base + 3584 + 256 * (u + 1)],
                        pdu[:, 256:512], cup[:])
                    nc.vector.tensor_add(
                        S[:, base + 1024 * u:base + 1024 * (u + 1)],
                        pj[:], CD12[:])
                    nc.scalar.copy(
                        out=S[:, base + 3072 + 256 * u:base + 3072 + 256 * (u + 1)],
                        in_=pdu[:, 0:256])
                    nc.scalar.copy(
                        out=S[:, base + 2048 + 512 * u:base + 2048 + 512 * (u + 1)],
                        in_=p3[:])
                    if u == 1:
                        nc.sync.dma_start(
                            out=slab[g][:, base:base + 4096],
                            in_=S[:, base:base + 4096])

    nc.compile()
    return nc


def _program() -> bass.Bass:
    global _PROGRAM
    if _PROGRAM is None:
        _PROGRAM = _build_program()
    return _PROGRAM


# --------------------------------------------------------------------------
# host entry point
# --------------------------------------------------------------------------

def _assemble(results):
    """8 per-core result dicts -> full [512, 512, 256] output."""
    out = np.empty((N, N, CH), np.float32)
    for k in range(NCORES):
        r = results[k]
        # halves h = sub//2: [g, p, h, {J12 2048, J3 1024, dg 512, up 512}]
        m = (np.asarray(r["slab"]).reshape(8, 128, 2, 4096)
             .astype(np.float32))
        slab = np.empty((RB, N, CH), np.float32)
        # main J blocks: [g, p, h, 1024*u + 512*Jh + 256*q + c]
        j12 = m[:, :, :, 0:2048].reshape(8, 128, 2, 2, 2, 2, CH)
        slab[:, 128:384, :] = (
            j12.transpose(0, 2, 3, 5, 4, 1, 6)    # g, h, u, q, Jh, p, c
            .reshape(RB, 2 * 128, CH))
        j3 = m[:, :, :, 2048:3072].reshape(8, 128, 2, 2, 2, CH)
        slab[:, 384:512, :] = (
            j3.transpose(0, 2, 3, 4, 1, 5).reshape(RB, 128, CH))
        # diag: [g, q*64+s, h, 3072 + 256*u + c] -> rows 8g+2sub+q, col s
        dg = m[:, :, :, 3072:3584].reshape(8, 2, 64, 2, 2, CH)
        slab[:, 0:64, :] = (
            dg.transpose(0, 3, 4, 1, 2, 5).reshape(RB, 64, CH))
        # upper: p<64 -> (q=0, s=64+p); p>=64 -> (q=1, s=p)
        up = m[:, :, :, 3584:4096].reshape(8, 2, 64, 2, 2, CH)
        slab[:, 64:128, :] = (
            up.transpose(0, 3, 4, 1, 2, 5).reshape(RB, 64, CH))
        base = RB * k
        out[base:base + RB] = np.roll(slab, base, axis=1)
    return out


def build_in_maps(x, W, b):
    shared = _shared_inputs(W, b)
    return [dict(shared, **_core_inputs(x, k)) for k in range(NCORES)]


def kernel(x, W, b):
    nc = _program()
    in_maps = build_in_maps(x, W, b)
    res = run_bass_kernel_spmd(nc, in_maps, core_ids=list(range(NCORES)))
    return _assemble(res.results)
